# revision 25
# baseline (speedup 1.0000x reference)
"""Trainium2 Bass kernel for nn_Grouping (segment_reduce / mean-pool by 4).

out[b, g, h] = sum_{j<4} feats[b, 4g+j, h] * values[b*S + 4g + j]

Sharding: data-parallel over B across 8 NeuronCores (2 batch elements per
core).  The segment-sum is fully local per core: each core streams its
[8192 tokens, 768] feats shard as 16 tiles of [128 groups, 4*768].

Memory-roofline design (timeline-sim 47.8 us/core vs 90.8 us for the f32
version; DMA engines are ~92% occupied, which is the floor for the bytes
moved):
  - Loads go through gpsimd (SWDGE) with an f32 -> bf16 cast in the DMA
    datapath, halving the SBUF-side DMA bytes (the dominant term).
  - The accumulator and the DRAM output are bf16; the host upcasts the
    gathered result to f32.  Measured end-to-end max rel err vs the f32
    reference is 5.2e-3 (harness gate is 2e-2).
  - Per tile the weighted 4-token sum runs as 4 tensor_scalar mults (4x
    DVE mode on packed bf16) + 3 tensor_tensor adds (2x mode), 2.4 us —
    the 1x-mode scalar_tensor_tensor chain costs 3.0 us and would pace
    the stream.
  - Every DMA has its own semaphore (16 load sems, 16 store sems): a wait
    of `sem >= 16` is exact per-DMA completion, immune to the
    cross-SDMA-engine skew that makes cumulative single-sem waits
    (`sem >= 16*(t+1)`) racy — the 16 SDMA engines increment
    independently, and one can run ahead to the next DMA before another
    finishes the current one.
  - Sub-tile splits (head/tail trims) were tried and measured slower: the
    DMA engines run gapless start-to-finish, so total time is
    head + total-DMA-busy + tail, and splits only add descriptor
    overhead (sub-512B descriptors transfer at half rate).
"""

import sys

import numpy as np

for _p in ("/opt/trn_rl_repo",):
    if _p not in sys.path:
        sys.path.insert(0, _p)

B, S, H = 16, 4096, 768
GROUP = 4
G = S // GROUP              # 1024 groups per batch element
NCORES = 8
B_PER = B // NCORES         # 2
TOK = B_PER * S             # 8192 tokens per core
GROUPS = B_PER * G          # 2048 groups per core
P = 128
NTILES = GROUPS // P        # 16
W = GROUP * H               # 3072 feats columns per tile row

_BUILT = None


def _build():
    """Build (once) the per-core Bass module. SPMD: identical on all cores."""
    global _BUILT
    if _BUILT is not None:
        return _BUILT

    import contextlib

    import concourse.bass as bass
    import concourse.mybir as mybir

    f32 = mybir.dt.float32
    bf16 = mybir.dt.bfloat16
    nc = bass.Bass(
        "TRN2",
        target_bir_lowering=False,
        debug=False,
        num_devices=NCORES,
    )

    feats = nc.dram_tensor("feats", [NTILES, P, GROUP, H], f32, kind="ExternalInput")
    # Host pre-transposed: vals[p, t*GROUP + j] = values[(t*P + p)*GROUP + j]
    vals = nc.dram_tensor("vals", [P, NTILES * GROUP], f32, kind="ExternalInput")
    out = nc.dram_tensor("out", [NTILES, P, H], bf16, kind="ExternalOutput")

    mult = mybir.AluOpType.mult
    add = mybir.AluOpType.add

    with contextlib.ExitStack() as ctx:
        xbuf = ctx.enter_context(nc.sbuf_tensor([P, NTILES, GROUP, H], bf16))
        obuf = ctx.enter_context(nc.sbuf_tensor([P, NTILES, H], bf16))
        ybuf = ctx.enter_context(nc.sbuf_tensor([P, 2 * H], bf16))
        vbuf = ctx.enter_context(nc.sbuf_tensor([P, NTILES * GROUP], f32))
        s_x = [
            ctx.enter_context(nc.semaphore(name=f"s_x{t}")) for t in range(NTILES)
        ]
        s_st = [
            ctx.enter_context(nc.semaphore(name=f"s_st{t}")) for t in range(NTILES)
        ]
        s_v = ctx.enter_context(nc.semaphore())
        s_cmp = ctx.enter_context(nc.semaphore())
        block = ctx.enter_context(nc.Block())

        # One unit per tile: (tile, ha, hb, load_sem), OPS_PER_TILE s_cmp
        # increments each.
        OPS_PER_TILE = 7
        units = [(t, 0, H, s_x[t]) for t in range(NTILES)]
        # Stores: (tile, ha, hb, n_units_done_before_firing, store_sem).
        stores = [(t, 0, H, t + 1, s_st[t]) for t in range(NTILES)]

        # gpsimd (SWDGE): casting loads (f32 -> bf16 in the DMA datapath),
        # one per unit, each with a dedicated semaphore.  No SBUF reuse
        # (all 16 tiles stay resident), so no waits at all.
        @block.gpsimd
        def _(gpsimd):
            for t, ha, hb, sem in units:
                gpsimd.dma_start(
                    out=xbuf[:, t, :, ha:hb], in_=feats[t][:, :, ha:hb]
                ).then_inc(sem, 16)

        # vector (DVE): per unit, out_bf16 = sum_j x_j * v_j, written as
        # 4 tensor_scalar mults (4x DVE mode on packed bf16) + 3 tensor_tensor
        # adds (2x mode) — 2.4 us/tile vs 3.0 us for the 1x-mode STT chain.
        # DVE is in-order, so the ybuf scratch slots need no cross-unit sync.
        @block.vector
        def _(vector):
            vector.wait_ge(s_v, 16)
            for t, ha, hb, sem in units:
                vector.wait_ge(sem, 16)
                o = obuf[:, t, ha:hb]
                n = hb - ha
                y1 = ybuf[:, 0:n]
                y2 = ybuf[:, H : H + n]

                def ts(dst, j, t=t, ha=ha, hb=hb):
                    vector.tensor_scalar(
                        dst, xbuf[:, t, j, ha:hb],
                        vbuf[:, GROUP * t + j : GROUP * t + j + 1], None, mult,
                    ).then_inc(s_cmp, 1)

                ts(o, 0)
                ts(y1, 1)
                vector.tensor_tensor(o, o, y1, add).then_inc(s_cmp, 1)
                ts(y1, 2)
                ts(y2, 3)
                vector.tensor_tensor(y1, y1, y2, add).then_inc(s_cmp, 1)
                vector.tensor_tensor(o, o, y1, add).then_inc(s_cmp, 1)

        # sync (HWDGE): vals load up front, then the stores.  Final waits
        # ensure the kernel isn't reported done with stores in flight (the
        # earlier sems are satisfied by then — only the last store's
        # completion is on the critical path).  Splitting stores across the
        # second HWDGE engine (Activation) was tried and measured slower
        # (ACT's DGE path is 784 ns vs SP's 650 ns and it landed the last
        # store later).
        @block.sync
        def _(sync):
            sync.dma_start(out=vbuf[:], in_=vals[:]).then_inc(s_v, 16)
            for t, ha, hb, ndone, sem in stores:
                sync.wait_ge(s_cmp, OPS_PER_TILE * ndone)
                sync.dma_start(
                    out=out[t][:, ha:hb], in_=obuf[:, t, ha:hb]
                ).then_inc(sem, 16)
            for _, _, _, _, sem in stores:
                sync.wait_ge(sem, 16)

    _BUILT = nc
    return nc


def _make_in_maps(feats, values):
    feats_sh = np.ascontiguousarray(feats, dtype=np.float32).reshape(
        NCORES, NTILES, P, GROUP, H
    )
    # [core, P, NTILES*GROUP] with vals[c, p, t*GROUP+j] = values shard token
    # (t*P + p)*GROUP + j — matches the kernel's "vals" layout.
    vals_sh = np.ascontiguousarray(
        np.asarray(values, dtype=np.float32)
        .reshape(NCORES, NTILES, P, GROUP)
        .transpose(0, 2, 1, 3)
        .reshape(NCORES, P, NTILES * GROUP)
    )
    return [{"feats": feats_sh[c], "vals": vals_sh[c]} for c in range(NCORES)]


def _run_on_device(feats, values, trace=False, **spmd_kwargs):
    """Shard inputs, run the SPMD kernel on 8 cores, gather full output.

    Returns (out [B, G, H] float32, BassKernelResults)."""
    from concourse.bass_utils import run_bass_kernel_spmd

    nc = _build()
    in_maps = _make_in_maps(feats, values)
    res = run_bass_kernel_spmd(
        nc, in_maps, list(range(NCORES)), trace=trace, **spmd_kwargs
    )
    full = np.stack(
        [np.asarray(res.results[c]["out"]) for c in range(NCORES)]
    ).astype(np.float32)
    return full.reshape(B, G, H), res


def _indices_match_structure(indices):
    """True iff indices encode the canonical grouping: token n = b*S + s with
    b = n // S, s = n % S, g = s // GROUP (the layout setup_inputs builds)."""
    idx = np.asarray(indices)
    if idx.shape != (3, B * S):
        return False
    n = np.arange(B * S, dtype=np.int64)
    return (
        np.array_equal(idx[0], n // S)
        and np.array_equal(idx[2], n % S)
        and np.array_equal(idx[1], (n % S) // GROUP)
    )


def kernel(feats, indices, values):
    if not _indices_match_structure(indices):
        # General (never hit for this problem's generator): numpy fallback.
        b_ids = np.asarray(indices[0], dtype=np.int64)
        g_ids = np.asarray(indices[1], dtype=np.int64)
        s_ids = np.asarray(indices[2], dtype=np.int64)
        gathered = np.asarray(feats)[b_ids, s_ids] * np.asarray(values)[:, None]
        out = np.zeros((B * G, feats.shape[-1]), dtype=np.float32)
        np.add.at(out, b_ids * G + g_ids, gathered)
        return out.reshape(B, G, feats.shape[-1])

    out, _ = _run_on_device(feats, values, trace=False)
    return out


# revision 37
# speedup vs baseline: 1.0079x; 1.0079x over previous
"""Trainium2 Bass kernel for nn_Grouping (segment_reduce / mean-pool by 4).

out[b, g, h] = sum_{j<4} feats[b, 4g+j, h] * values[b*S + 4g + j]

Sharding: data-parallel over B across 8 NeuronCores (2 batch elements per
core).  The segment-sum is fully local per core: each core streams its
[8192 tokens, 768] feats shard as 16 tiles of [128 groups, 4*768].

Memory-roofline design (timeline-sim 47.6 us/core on the uniform-values
fast path, 47.8 us general, vs 90.8 us for the f32 version; the DMA
engines run gapless start-to-finish, which is the floor for the bytes
moved):
  - Loads go through gpsimd (SWDGE) with an f32 -> bf16 cast in the DMA
    datapath, halving the SBUF-side DMA bytes (the dominant term).
  - The accumulator and the DRAM output are bf16; the host upcasts the
    gathered result to f32.  Measured end-to-end max rel err vs the f32
    reference is 5.2e-3 (harness gate is 2e-2).
  - When values are uniform (this generator: 1/GROUP everywhere), the
    weight folds into an immediate: no vals DMA, and each tile is 3
    tensor_tensor adds (2x DVE mode) + 1 tensor_scalar (4x mode).  The
    general path streams vals and runs 4 tensor_scalar + 3 tensor_tensor
    per tile — either way far below the 1x-mode scalar_tensor_tensor
    chain that would pace the stream.
  - Every DMA has its own semaphore (16 load sems, 16 store sems): a wait
    of `sem >= 16` is exact per-DMA completion, immune to the
    cross-SDMA-engine skew that makes cumulative single-sem waits
    (`sem >= 16*(t+1)`) racy — the 16 SDMA engines increment
    independently, and one can run ahead to the next DMA before another
    finishes the current one.
  - Measured-and-rejected in the simulator: sub-tile head/tail splits
    (only add descriptor overhead; sub-512B descriptors transfer at half
    rate), dual-engine stores (ACT's DGE path is slower and lands the
    last store later), no_gpsimd_drain, DMA-accumulate variants (same
    charged dest bytes, plus 1 us SWDGE descgen per extra DMA).
"""

import sys

import numpy as np

for _p in ("/opt/trn_rl_repo",):
    if _p not in sys.path:
        sys.path.insert(0, _p)

B, S, H = 16, 4096, 768
GROUP = 4
G = S // GROUP              # 1024 groups per batch element
NCORES = 8
B_PER = B // NCORES         # 2
TOK = B_PER * S             # 8192 tokens per core
GROUPS = B_PER * G          # 2048 groups per core
P = 128
NTILES = GROUPS // P        # 16
W = GROUP * H               # 3072 feats columns per tile row

_BUILT = {}


def _build(scale=None):
    """Build (once per variant) the per-core Bass module. SPMD: identical on
    all cores.

    scale=None: general kernel — per-token weights stream in as the `vals`
    input and each tile costs 4 tensor_scalar + 3 tensor_tensor DVE ops.
    scale=<float>: uniform-values fast path — the weight is baked in as an
    immediate, the vals DMA disappears, and each tile is 3 adds + 1 scale.
    """
    key = None if scale is None else float(scale)
    if key in _BUILT:
        return _BUILT[key]

    import contextlib

    import concourse.bass as bass
    import concourse.mybir as mybir

    f32 = mybir.dt.float32
    bf16 = mybir.dt.bfloat16
    nc = bass.Bass(
        "TRN2",
        target_bir_lowering=False,
        debug=False,
        num_devices=NCORES,
    )

    feats = nc.dram_tensor("feats", [NTILES, P, W], f32, kind="ExternalInput")
    if scale is None:
        # Host pre-transposed: vals[p, t*GROUP+j] = values[(t*P+p)*GROUP+j]
        vals = nc.dram_tensor("vals", [P, NTILES * GROUP], f32, kind="ExternalInput")
    out = nc.dram_tensor("out", [NTILES, P, H], bf16, kind="ExternalOutput")

    mult = mybir.AluOpType.mult
    add = mybir.AluOpType.add

    # Head-window prefetch (uniform path only): the DMA engines idle in
    # [~2.33us, ~2.74us] — SP's HWDGE pipeline is ready before Pool's SWDGE
    # descgen.  SP prefetches the first HPRE columns of tile 0 token 0 as
    # raw f32 into a staging buffer during that window (f32 because HWDGE
    # cannot cast); the Pool bf16 stream shrinks by those bytes.  HPRE is
    # sized so the f32 transfer (2x the bf16 cost) still fits inside the
    # idle window.
    HPRE = 284 if scale is not None else 0

    with contextlib.ExitStack() as ctx:
        xbuf = ctx.enter_context(nc.sbuf_tensor([P, NTILES, W], bf16))
        obuf = ctx.enter_context(nc.sbuf_tensor([P, NTILES, H], bf16))
        ybuf = ctx.enter_context(nc.sbuf_tensor([P, 2 * H], bf16))
        if HPRE:
            xf32 = ctx.enter_context(nc.sbuf_tensor([P, HPRE], f32))
            s_f = ctx.enter_context(nc.semaphore(name="s_f"))
        if scale is None:
            vbuf = ctx.enter_context(nc.sbuf_tensor([P, NTILES * GROUP], f32))
        s_x = [
            ctx.enter_context(nc.semaphore(name=f"s_x{t}")) for t in range(NTILES)
        ]
        s_st = [
            ctx.enter_context(nc.semaphore(name=f"s_st{t}")) for t in range(NTILES)
        ]
        s_v = ctx.enter_context(nc.semaphore())
        s_cmp = ctx.enter_context(nc.semaphore())
        block = ctx.enter_context(nc.Block())

        OPS_PER_TILE = 7 if scale is None else 4
        # s_cmp value after tile t's compute (tile 0 costs extra ops when the
        # prefetch splits its token-0 row).
        TILE0_OPS = OPS_PER_TILE + (4 if HPRE else 0)
        cmp_after = [TILE0_OPS + OPS_PER_TILE * t for t in range(NTILES)]

        # gpsimd (SWDGE): casting loads (f32 -> bf16 in the DMA datapath),
        # one per tile, each with a dedicated semaphore.  No SBUF reuse
        # (all 16 tiles stay resident), so no waits at all.  With the
        # prefetch, tile 0's gpsimd DMA covers only columns [HPRE:W].
        @block.gpsimd
        def _(gpsimd):
            # Tile 0's remainder after the prefetched sliver is contiguous in
            # the flat per-tile layout, so it stays a single DMA (a second
            # short DMA would expose the serial ~1.7us descgen+DGE latency
            # as a stream gap).
            gpsimd.dma_start(
                out=xbuf[:, 0, HPRE:W], in_=feats[0][:, HPRE:W]
            ).then_inc(s_x[0], 16)
            for t in range(1, NTILES):
                gpsimd.dma_start(out=xbuf[:, t], in_=feats[t]).then_inc(
                    s_x[t], 16
                )

        # vector (DVE): per tile, out_bf16 = sum_j x_j * v_j, using only
        # 4x-mode tensor_scalar (packed bf16) and 2x-mode tensor_tensor —
        # the 1x-mode scalar_tensor_tensor chain would pace the stream.
        # DVE is in-order, so the ybuf scratch slots need no cross-tile sync.
        @block.vector
        def _(vector):
            def sum4(o, x0, x1, x2, x3, y1, y2):
                vector.tensor_tensor(y1, x0, x1, add).then_inc(s_cmp, 1)
                vector.tensor_tensor(y2, x2, x3, add).then_inc(s_cmp, 1)
                vector.tensor_tensor(y1, y1, y2, add).then_inc(s_cmp, 1)
                vector.tensor_scalar(
                    o, y1, float(scale), None, mult
                ).then_inc(s_cmp, 1)

            if scale is None:
                vector.wait_ge(s_v, 16)
            for t in range(NTILES):
                if t == 0 and HPRE:
                    vector.wait_ge(s_f, 16)
                vector.wait_ge(s_x[t], 16)
                o = obuf[:, t]
                y1 = ybuf[:, 0:H]
                y2 = ybuf[:, H : 2 * H]
                x = [xbuf[:, t, j * H : (j + 1) * H] for j in range(GROUP)]

                if scale is None:
                    def ts(dst, j, t=t):
                        vector.tensor_scalar(
                            dst, xbuf[:, t, j * H : (j + 1) * H],
                            vbuf[:, GROUP * t + j : GROUP * t + j + 1],
                            None, mult,
                        ).then_inc(s_cmp, 1)

                    ts(o, 0)
                    ts(y1, 1)
                    vector.tensor_tensor(o, o, y1, add).then_inc(s_cmp, 1)
                    ts(y1, 2)
                    ts(y2, 3)
                    vector.tensor_tensor(y1, y1, y2, add).then_inc(s_cmp, 1)
                    vector.tensor_tensor(o, o, y1, add).then_inc(s_cmp, 1)
                elif t == 0 and HPRE:
                    # token-0 columns [0:HPRE] came in as raw f32 (the SP
                    # head-window prefetch); DVE mixes dtypes at the ports.
                    sum4(o[:, 0:HPRE], xf32[:],
                         x[1][:, 0:HPRE], x[2][:, 0:HPRE], x[3][:, 0:HPRE],
                         ybuf[:, 0:HPRE], ybuf[:, H : H + HPRE])
                    sum4(o[:, HPRE:H],
                         x[0][:, HPRE:H], x[1][:, HPRE:H],
                         x[2][:, HPRE:H], x[3][:, HPRE:H],
                         ybuf[:, HPRE:H], ybuf[:, H + HPRE : 2 * H])
                else:
                    sum4(o, x[0], x[1], x[2], x[3], y1, y2)

        # sync (HWDGE): vals load up front (general path only), then the
        # stores.  Final waits ensure the kernel isn't reported done with
        # stores in flight (the earlier sems are satisfied by then — only
        # the last store's completion is on the critical path).  Splitting
        # stores across the second HWDGE engine (Activation) was tried and
        # measured slower (ACT's DGE path is 784 ns vs SP's 650 ns and it
        # landed the last store later).
        @block.sync
        def _(sync):
            if HPRE:
                sync.dma_start(
                    out=xf32[:], in_=feats[0][:, 0:HPRE]
                ).then_inc(s_f, 16)
            if scale is None:
                sync.dma_start(out=vbuf[:], in_=vals[:]).then_inc(s_v, 16)
            for t in range(NTILES):
                sync.wait_ge(s_cmp, cmp_after[t])
                sync.dma_start(out=out[t], in_=obuf[:, t]).then_inc(s_st[t], 16)
            for t in range(NTILES):
                sync.wait_ge(s_st[t], 16)

    _BUILT[key] = nc
    return nc


def _make_in_maps(feats, values=None):
    feats_sh = np.ascontiguousarray(feats, dtype=np.float32).reshape(
        NCORES, NTILES, P, W
    )
    if values is None:
        return [{"feats": feats_sh[c]} for c in range(NCORES)]
    # [core, P, NTILES*GROUP] with vals[c, p, t*GROUP+j] = values shard token
    # (t*P + p)*GROUP + j — matches the kernel's "vals" layout.
    vals_sh = np.ascontiguousarray(
        np.asarray(values, dtype=np.float32)
        .reshape(NCORES, NTILES, P, GROUP)
        .transpose(0, 2, 1, 3)
        .reshape(NCORES, P, NTILES * GROUP)
    )
    return [{"feats": feats_sh[c], "vals": vals_sh[c]} for c in range(NCORES)]


def _run_on_device(feats, values, scale=None, trace=False, **spmd_kwargs):
    """Shard inputs, run the SPMD kernel on 8 cores, gather full output.

    scale=<float> selects the uniform-values fast path (values not shipped).
    Returns (out [B, G, H] float32, BassKernelResults)."""
    from concourse.bass_utils import run_bass_kernel_spmd

    nc = _build(scale)
    in_maps = _make_in_maps(feats, None if scale is not None else values)
    res = run_bass_kernel_spmd(
        nc, in_maps, list(range(NCORES)), trace=trace, **spmd_kwargs
    )
    full = np.stack(
        [np.asarray(res.results[c]["out"]) for c in range(NCORES)]
    ).astype(np.float32)
    return full.reshape(B, G, H), res


def _indices_match_structure(indices):
    """True iff indices encode the canonical grouping: token n = b*S + s with
    b = n // S, s = n % S, g = s // GROUP (the layout setup_inputs builds)."""
    idx = np.asarray(indices)
    if idx.shape != (3, B * S):
        return False
    n = np.arange(B * S, dtype=np.int64)
    return (
        np.array_equal(idx[0], n // S)
        and np.array_equal(idx[2], n % S)
        and np.array_equal(idx[1], (n % S) // GROUP)
    )


def kernel(feats, indices, values):
    if not _indices_match_structure(indices):
        # General (never hit for this problem's generator): numpy fallback.
        b_ids = np.asarray(indices[0], dtype=np.int64)
        g_ids = np.asarray(indices[1], dtype=np.int64)
        s_ids = np.asarray(indices[2], dtype=np.int64)
        gathered = np.asarray(feats)[b_ids, s_ids] * np.asarray(values)[:, None]
        out = np.zeros((B * G, feats.shape[-1]), dtype=np.float32)
        np.add.at(out, b_ids * G + g_ids, gathered)
        return out.reshape(B, G, feats.shape[-1])

    v = np.asarray(values, dtype=np.float32)
    scale = float(v.flat[0]) if v.size and np.all(v == v.flat[0]) else None
    out, _ = _run_on_device(feats, values, scale=scale, trace=False)
    return out
